# revision 11
# baseline (speedup 1.0000x reference)
"""Memory-efficient multi-head attention on 8 Trainium2 NeuronCores.

Sharding: tensor-parallel over heads (4 head-groups) x data-parallel over
batch (2) = 8 cores. Core c handles head group g = c % 4 (heads 4g..4g+3,
feature slice 512) of batch b = c // 4. Each core computes its Q/K/V
projections from sliced weights, attention for its 4 heads, and a partial
out-projection y_c = ao_c @ Wo[:, gs].T; the host sums the 4 partials per
batch and adds the output bias.

Bias algebra: softmax is invariant to per-row constants, so the K bias is
dropped entirely (only Q keeps its bias); since attention rows sum to 1,
the V bias is a constant folded into the host-side bias (bv @ Wo.T + bo).

Inputs stream in bf16 (PE runs bf16 at full rate); PSUM accumulates fp32.
Softmax denominators are accumulated on DVE+Pool, reduced over partitions
by two tiny ones-matmuls on the PE, inverted with the fast approximate
reciprocal, and broadcast on Pool.

Schedule: K projection first, then Q+V per seq block with three score/exp
stages prepaid in between (so the Act engine's exp stream starts early),
then a software pipeline of B(g) attention-value stages and A(g+3) score
stages with the out-projection interleaved per q block.
"""

import sys

if "/opt/trn_rl_repo" not in sys.path:
    sys.path.insert(0, "/opt/trn_rl_repo")

from contextlib import ExitStack

import ml_dtypes
import numpy as np

import concourse.bacc as bacc
import concourse.mybir as mybir
import concourse.tile as tile
from concourse.bass_utils import run_bass_kernel_spmd

B, S, D, H = 2, 2048, 2048, 16
HD = 128               # head dim
G = 4                  # head groups (tensor-parallel degree)
HPG = H // G           # heads per group = 4
FC = HPG * HD          # per-core feature slice = 512
KC = D // 128          # contraction chunks = 16
SB = 4                 # seq blocks (512 cols each)
QB = 4                 # q blocks (512 each)
ST = S // 128          # seq tiles = 16
SCALE = float(HD) ** -0.5
PRE_UNMASKED = 3       # prepaid A stages (et ring depth)

F32 = mybir.dt.float32
F32R = mybir.dt.float32r
BF16 = mybir.dt.bfloat16
NPBF16 = ml_dtypes.bfloat16

PROFILE = False        # set by test.py to collect an NTFF trace
LAST = {}              # exec_time_ns etc. stashed here when PROFILE

_cache = {}


def _build(masked: bool):
    PRE = 2 if masked else PRE_UNMASKED
    nc = bacc.Bacc("TRN2", target_bir_lowering=False)

    xT = nc.dram_tensor("xT", (D, S), BF16, kind="ExternalInput")
    wqT = nc.dram_tensor("wqT", (D, FC), BF16, kind="ExternalInput")
    wkT = nc.dram_tensor("wkT", (D, FC), BF16, kind="ExternalInput")
    wvT = nc.dram_tensor("wvT", (D, FC), BF16, kind="ExternalInput")
    woT = nc.dram_tensor("woT", (FC, D), BF16, kind="ExternalInput")
    bqb = nc.dram_tensor("bqb", (128, HPG, 512), F32, kind="ExternalInput")
    ones_k = nc.dram_tensor("ones_k", (128, 1), F32R, kind="ExternalInput")
    mT = None
    if masked:
        mT = nc.dram_tensor("mT", (S, S), F32, kind="ExternalInput")
    y = nc.dram_tensor("y", (S, D), F32, kind="ExternalOutput")

    # DRAM views tiled to 128 partitions: (c p) f -> p c f
    xT_v = xT[:].rearrange("(c p) s -> p c s", p=128)
    wqT_v = wqT[:].rearrange("(c p) f -> p c f", p=128)
    wkT_v = wkT[:].rearrange("(c p) f -> p c f", p=128)
    wvT_v = wvT[:].rearrange("(c p) f -> p c f", p=128)
    woT_v = woT[:].rearrange("(c p) f -> p c f", p=128)
    mT_v = mT[:].rearrange("(c p) q -> p c q", p=128) if masked else None

    with tile.TileContext(nc) as tc, ExitStack() as top:
        const = top.enter_context(tc.tile_pool(name="const", bufs=1))
        persist = top.enter_context(tc.tile_pool(name="persist", bufs=1))
        p2s = top.enter_context(tc.tile_pool(name="p2s", bufs=1))
        p2ps = top.enter_context(
            tc.tile_pool(name="p2ps", bufs=1, space="PSUM")
        )

        t_bqb = const.tile([128, HPG, 512], F32, tag="bqb")
        t_ok = const.tile([128, 1], F32R, tag="ok")

        QT = [persist.tile([128, S], BF16, tag=f"qt{h}", name=f"qt{h}") for h in range(HPG)]
        KT = [persist.tile([128, S], BF16, tag=f"kt{h}", name=f"kt{h}") for h in range(HPG)]
        V = [persist.tile([128, FC], BF16, tag=f"v{st}", name=f"v{st}") for st in range(ST)]
        aoT = []
        WoS = []

        ET = {}  # (h, qb) -> (et tiles, accd, accp)
        PB = [None]  # phase-2/3 PSUM pool, opened after phase 1

        def scores(h, qsl, ktp):
            t = p2ps.tile([128, 1024], F32, tag="s", bufs=2, name="ps_s")
            for half in range(2):
                kt = 2 * ktp + half
                nc.tensor.matmul(
                    t[:, half * 512 : (half + 1) * 512],
                    KT[h][:, kt * 128 : (kt + 1) * 128],
                    QT[h][:, qsl],
                    start=True,
                    stop=True,
                )
            return t

        def stage_a(h, qb):
            """Scores + exp + denominator chunk-accumulate for (h, qb)."""
            qsl = slice(qb * 512, (qb + 1) * 512)
            accd = p2s.tile([128, 512], F32R, tag="accd", bufs=PRE + 1, name="accd")
            accp = p2s.tile([128, 512], F32R, tag="accp", bufs=PRE + 1, name="accp")
            ets = []
            cur = scores(h, qsl, 0)
            for ktp in range(8):
                nxt = scores(h, qsl, ktp + 1) if ktp < 7 else None
                et = p2s.tile([128, 1024], BF16, tag=f"et{ktp}", bufs=PRE, name=f"et{ktp}")
                nc.scalar.activation(
                    et[:],
                    cur[:],
                    mybir.ActivationFunctionType.Exp,
                    scale=SCALE,
                )
                if masked:
                    mtile = p2s.tile([128, 2, 512], F32, tag="mtile", bufs=3, name="mtile")
                    nc.scalar.dma_start(
                        mtile[:], mT_v[:, 2 * ktp : 2 * ktp + 2, qsl]
                    )
                    nc.vector.tensor_mul(
                        et[:],
                        et[:],
                        mtile[:].rearrange("p c q -> p (c q)"),
                    )
                eng, acc = (nc.vector, accd) if ktp < 4 else (nc.gpsimd, accp)
                if ktp in (0, 4):
                    eng.tensor_add(acc[:], et[:, 0:512], et[:, 512:1024])
                else:
                    eng.tensor_add(acc[:], acc[:], et[:, 0:512])
                    eng.tensor_add(acc[:], acc[:], et[:, 512:1024])
                ets.append(et)
                cur = nxt
            ET[(h, qb)] = (ets, accd, accp)

        def stage_b(h, qb):
            """AV matmuls + denominator reduce + normalize into aoT."""
            qsl = slice(qb * 512, (qb + 1) * 512)
            ets, accd, accp = ET.pop((h, qb))
            ps_av = PB[0].tile([128, 512], F32, tag="av", bufs=2, name="ps_av")
            for ktp in range(8):
                for half in range(2):
                    kt = 2 * ktp + half
                    esl = slice(half * 512, (half + 1) * 512)
                    nc.tensor.matmul(
                        ps_av[:],
                        V[kt][:, h * 128 : (h + 1) * 128],
                        ets[ktp][:, esl],
                        start=(kt == 0),
                        stop=(kt == ST - 1),
                    )
            # denominator pairs with ps_av in the "av" ring: both slots'
            # consumers (recip, mul) complete within the same group, so no
            # cross-round convoy through this ring
            psd_t = PB[0].tile([128, 512], F32, tag="av", bufs=2, name="psd_t")
            psd = psd_t[0:1, :]
            nc.tensor.matmul(psd, t_ok[:], accd[:], start=True, stop=False)
            nc.tensor.matmul(psd, t_ok[:], accp[:], start=False, stop=True)
            rs = p2s.tile([1, 512], F32, tag="rs", bufs=2, name="rs")
            nc.vector.reciprocal_approx_fast(rs[:], psd)
            bc = p2s.tile([128, 512], F32, tag="bc", bufs=2, name="bc")
            nc.gpsimd.partition_broadcast(bc[:], rs[:])
            nc.vector.tensor_mul(aoT[h][:, qsl], ps_av[:], bc[:])

        def out_proj(qb, p3s):
            """Out-projection + store for the 4 seq tiles of q block qb."""
            for t in range(4):
                st = qb * 4 + t
                stsl = slice(st * 128, (st + 1) * 128)
                yt = p3s.tile([128, D], F32, tag="yt", bufs=2, name="yt")
                for dcb in range(4):
                    dsl = slice(dcb * 512, (dcb + 1) * 512)
                    psy = PB[0].tile([128, 512], F32, tag="pmix", bufs=2, name="psy")
                    for fc in range(HPG):
                        nc.tensor.matmul(
                            psy[:],
                            aoT[fc][:, stsl],
                            WoS[fc][:, dsl],
                            start=(fc == 0),
                            stop=(fc == HPG - 1),
                        )
                    if dcb < 3:
                        nc.vector.tensor_copy(yt[:, dsl], psy[:])
                    else:
                        nc.scalar.copy(yt[:, dsl], psy[:])
                qeng = nc.sync if st % 2 == 0 else nc.gpsimd
                qeng.dma_start(y[stsl, 0:1024], yt[:, 0:1024])
                qeng.dma_start(y[stsl, 1024:2048], yt[:, 1024:2048])

        # ---- phase 1: projections (K pass, then Q+V per seq block) ----
        with tc.tile_pool(name="p1x", bufs=1) as p1x, \
             tc.tile_pool(name="p1w", bufs=1) as p1w, \
             tc.tile_pool(name="p1ps", bufs=1, space="PSUM") as p1ps:
            t_wk = p1w.tile([128, KC, FC], BF16, tag="wk")
            t_wq = p1w.tile([128, KC, FC], BF16, tag="wq")
            t_wv = p1w.tile([128, KC, FC], BF16, tag="wv")
            for wt, wv_ in ((t_wk, wkT_v), (t_wq, wqT_v), (t_wv, wvT_v)):
                for ck in range(4):
                    nc.sync.dma_start(
                        wt[:, 4 * ck : 4 * ck + 4, :],
                        wv_[:, 4 * ck : 4 * ck + 4, :],
                    )

            def xload(sb):
                ssl = slice(sb * 512, (sb + 1) * 512)
                halves = []
                for hh in range(2):
                    xb = p1x.tile(
                        [128, 8, 512], BF16, tag="xb", bufs=3, name="xb"
                    )
                    for xc, eng in ((0, nc.scalar), (1, nc.gpsimd)):
                        c0 = 8 * hh + 4 * xc
                        eng.dma_start(
                            xb[:, 4 * xc : 4 * xc + 4, :],
                            xT_v[:, c0 : c0 + 4, ssl],
                        )
                    halves.append(xb)
                return halves

            # K pass (no bias needed: softmax is row-shift invariant)
            for sb in range(SB):
                ssl = slice(sb * 512, (sb + 1) * 512)
                xb = xload(sb)
                ps = [
                    p1ps.tile([128, 512], F32, tag="p1", bufs=4, name=f"p1_{mt}")
                    for mt in range(HPG)
                ]
                for kc in range(KC):
                    for mt in range(HPG):
                        nc.tensor.matmul(
                            ps[mt][:],
                            t_wk[:, kc, mt * 128 : (mt + 1) * 128],
                            xb[kc // 8][:, kc % 8, :],
                            start=(kc == 0),
                            stop=(kc == KC - 1),
                        )
                for mt in range(HPG):
                    nc.scalar.copy(KT[mt][:, ssl], ps[mt][:])

            # Q + V pass, with prepaid A stages between blocks
            nc.gpsimd.dma_start(t_bqb[:], bqb[:])
            nc.gpsimd.dma_start(t_ok[:], ones_k[:])
            for sb in range(SB):
                ssl = slice(sb * 512, (sb + 1) * 512)
                xb = xload(sb)
                ps = [
                    p1ps.tile([128, 512], F32, tag="p1", bufs=4, name=f"p1_{mt}")
                    for mt in range(HPG)
                ]
                for kc in range(KC):
                    for mt in range(HPG):
                        nc.tensor.matmul(
                            ps[mt][:],
                            t_wq[:, kc, mt * 128 : (mt + 1) * 128],
                            xb[kc // 8][:, kc % 8, :],
                            start=(kc == 0),
                            stop=(kc == KC - 1),
                        )
                for mt in range(HPG):
                    nc.vector.tensor_add(
                        QT[mt][:, ssl], ps[mt][:], t_bqb[:, mt, :]
                    )
                psv = [
                    p1ps.tile([128, 512], F32, tag="p1", bufs=4, name=f"p1v{j}")
                    for j in range(4)
                ]
                for kc in range(KC):
                    for j in range(4):
                        nc.tensor.matmul(
                            psv[j][:],
                            xb[kc // 8][:, kc % 8, j * 128 : (j + 1) * 128],
                            t_wv[:, kc, :],
                            start=(kc == 0),
                            stop=(kc == KC - 1),
                        )
                for j in range(4):
                    nc.vector.tensor_copy(V[sb * 4 + j][:], psv[j][:])
                if sb < PRE:
                    stage_a(sb, 0)

        # ---- phase 2+3: pipelined attention + out-projection ----
        with tc.tile_pool(name="p23", bufs=1) as p23, \
             tc.tile_pool(name="p23ps", bufs=1, space="PSUM") as p23ps, \
             tc.tile_pool(name="p3s", bufs=1) as p3s:
            PB[0] = p23ps
            aoT.extend(
                p23.tile([128, S], BF16, tag=f"ao{h}", name=f"ao{h}")
                for h in range(HPG)
            )
            WoS.extend(
                p23.tile([128, D], BF16, tag=f"wo{fc}", name=f"wo{fc}")
                for fc in range(HPG)
            )
            for fc in range(HPG):
                nc.sync.dma_start(WoS[fc][:], woT_v[:, fc, :])

            groups = [(h, qb) for qb in range(QB) for h in range(HPG)]
            for i, g in enumerate(groups):
                stage_b(*g)
                if i + PRE < len(groups):
                    stage_a(*groups[i + PRE])
                # out-projection for qb, two groups after its last head so
                # the normalize chain for (h3, qb) is never on the PE's
                # critical path
                if i >= 5 and (i - 5) % HPG == 0:
                    out_proj((i - 5) // HPG, p3s)
            out_proj(QB - 1, p3s)

    nc.finalize()
    return nc


def kernel(x, mask, Wq, bq, Wk, bk, Wv, bv, Wo, bo):
    x = np.asarray(x, dtype=np.float32)
    mask = np.asarray(mask)
    Wq, bq = np.asarray(Wq, np.float32), np.asarray(bq, np.float32)
    Wk = np.asarray(Wk, np.float32)
    Wv, bv = np.asarray(Wv, np.float32), np.asarray(bv, np.float32)
    Wo, bo = np.asarray(Wo, np.float32), np.asarray(bo, np.float32)

    masked = bool((mask == 0).any())
    key = masked
    if key not in _cache:
        _cache[key] = _build(masked)
    nc = _cache[key]

    xTb = [np.ascontiguousarray(x[b].T).astype(NPBF16) for b in range(B)]
    mTb = None
    if masked:
        mTb = [
            np.ascontiguousarray((mask[b, 0] != 0).T.astype(np.float32))
            for b in range(B)
        ]
    ok = np.ones((128, 1), np.float32)

    in_maps = []
    for c in range(8):
        g, b = c % G, c // G
        gs = slice(g * FC, (g + 1) * FC)
        bq2 = np.ascontiguousarray(bq[gs].reshape(HPG, 128).T)
        m = {
            "xT": xTb[b],
            "wqT": np.ascontiguousarray(Wq[gs].T).astype(NPBF16),
            "wkT": np.ascontiguousarray(Wk[gs].T).astype(NPBF16),
            "wvT": np.ascontiguousarray(Wv[gs].T).astype(NPBF16),
            "woT": np.ascontiguousarray(Wo[:, gs].T).astype(NPBF16),
            "bqb": np.ascontiguousarray(
                np.broadcast_to(bq2[:, :, None], (128, HPG, 512))
            ).astype(np.float32),
            "ones_k": ok,
        }
        if masked:
            m["mT"] = mTb[b]
        in_maps.append(m)

    res = run_bass_kernel_spmd(
        nc, in_maps, core_ids=list(range(8)), trace=PROFILE
    )
    if PROFILE:
        LAST["exec_time_ns"] = res.exec_time_ns
        LAST["profile_json"] = res.profile_json
        LAST["trace"] = res.instructions_and_trace

    # host-side bias: bo plus the folded V bias (rows of attn sum to 1)
    bias = bo.astype(np.float64) + bv.astype(np.float64) @ Wo.T.astype(
        np.float64
    )
    out = np.empty((B, S, D), np.float32)
    for b in range(B):
        acc = res.results[4 * b]["y"].astype(np.float64)
        for g in range(1, G):
            acc += res.results[4 * b + g]["y"]
        out[b] = (acc + bias).astype(np.float32)
    return out


# revision 13
# speedup vs baseline: 1.2654x; 1.2654x over previous
"""Memory-efficient multi-head attention on 8 Trainium2 NeuronCores.

Sharding: tensor-parallel over heads (4 head-groups) x data-parallel over
batch (2) = 8 cores. Core c handles head group g = c % 4 (heads 4g..4g+3,
feature slice 512) of batch b = c // 4. Each core computes its Q/K/V
projections from sliced weights, attention for its 4 heads, and a partial
out-projection y_c = ao_c @ Wo[:, gs].T; the host sums the 4 partials per
batch and adds the output bias.

Bias algebra: softmax is invariant to per-row constants, so the K bias is
dropped entirely (only Q keeps its bias); since attention rows sum to 1,
the V bias is a constant folded into the host-side bias (bv @ Wo.T + bo).

Inputs stream in bf16 (PE runs bf16 at full rate); PSUM accumulates fp32.
Softmax denominators are accumulated on DVE+Pool, reduced over partitions
by two tiny ones-matmuls on the PE, inverted with the fast approximate
reciprocal, and broadcast on Pool.

Schedule: K projection first, then Q+V per seq block with three score/exp
stages prepaid in between (so the Act engine's exp stream starts early),
then a software pipeline of B(g) attention-value stages and A(g+3) score
stages with the out-projection interleaved per q block.
"""

import sys

if "/opt/trn_rl_repo" not in sys.path:
    sys.path.insert(0, "/opt/trn_rl_repo")

from contextlib import ExitStack

import ml_dtypes
import numpy as np

import concourse.bacc as bacc
import concourse.mybir as mybir
import concourse.tile as tile
from concourse.bass_utils import run_bass_kernel_spmd

B, S, D, H = 2, 2048, 2048, 16
HD = 128               # head dim
G = 4                  # head groups (tensor-parallel degree)
HPG = H // G           # heads per group = 4
FC = HPG * HD          # per-core feature slice = 512
KC = D // 128          # contraction chunks = 16
SB = 4                 # seq blocks (512 cols each)
QB = 4                 # q blocks (512 each)
ST = S // 128          # seq tiles = 16
SCALE = float(HD) ** -0.5
PRE_UNMASKED = 3       # prepaid A stages (et ring depth)

F32 = mybir.dt.float32
F32R = mybir.dt.float32r
BF16 = mybir.dt.bfloat16
NPBF16 = ml_dtypes.bfloat16

PROFILE = False        # set by test.py to collect an NTFF trace
LAST = {}              # exec_time_ns etc. stashed here when PROFILE

_cache = {}


def _build(masked: bool):
    PRE = 2 if masked else PRE_UNMASKED
    nc = bacc.Bacc("TRN2", target_bir_lowering=False)

    xT = nc.dram_tensor("xT", (D, S), BF16, kind="ExternalInput")
    wqT = nc.dram_tensor("wqT", (D, FC), BF16, kind="ExternalInput")
    wkT = nc.dram_tensor("wkT", (D, FC), BF16, kind="ExternalInput")
    wvT = nc.dram_tensor("wvT", (D, FC), BF16, kind="ExternalInput")
    woT = nc.dram_tensor("woT", (FC, D), BF16, kind="ExternalInput")
    bqb = nc.dram_tensor("bqb", (128, HPG, 512), F32, kind="ExternalInput")
    ones_k = nc.dram_tensor("ones_k", (128, 1), F32R, kind="ExternalInput")
    ones_kb = nc.dram_tensor("ones_kb", (128, 1), BF16, kind="ExternalInput")
    mT = None
    if masked:
        mT = nc.dram_tensor("mT", (S, S), F32, kind="ExternalInput")
    y = nc.dram_tensor("y", (S, D), F32, kind="ExternalOutput")

    # DRAM views tiled to 128 partitions: (c p) f -> p c f
    xT_v = xT[:].rearrange("(c p) s -> p c s", p=128)
    wqT_v = wqT[:].rearrange("(c p) f -> p c f", p=128)
    wkT_v = wkT[:].rearrange("(c p) f -> p c f", p=128)
    wvT_v = wvT[:].rearrange("(c p) f -> p c f", p=128)
    woT_v = woT[:].rearrange("(c p) f -> p c f", p=128)
    mT_v = mT[:].rearrange("(c p) q -> p c q", p=128) if masked else None

    with tile.TileContext(nc) as tc, ExitStack() as top:
        const = top.enter_context(tc.tile_pool(name="const", bufs=1))
        persist = top.enter_context(tc.tile_pool(name="persist", bufs=1))
        p2s = top.enter_context(tc.tile_pool(name="p2s", bufs=1))
        p2ps = top.enter_context(
            tc.tile_pool(name="p2ps", bufs=1, space="PSUM")
        )

        t_bqb = const.tile([128, HPG, 512], F32, tag="bqb")
        t_ok = const.tile([128, 1], F32R, tag="ok")
        t_okb = const.tile([128, 1], BF16, tag="okb")

        QT = [persist.tile([128, S], BF16, tag=f"qt{h}", name=f"qt{h}") for h in range(HPG)]
        KT = [persist.tile([128, S], BF16, tag=f"kt{h}", name=f"kt{h}") for h in range(HPG)]
        V = [persist.tile([128, FC], BF16, tag=f"v{st}", name=f"v{st}") for st in range(ST)]
        aoT = []
        WoS = []

        ET = {}  # (h, qb) -> (et tiles, accd, accp)
        PB = [None]  # phase-2/3 PSUM pool, opened after phase 1

        def scores(h, qsl, ktp):
            t = p2ps.tile([128, 1024], F32, tag="s", bufs=2, name="ps_s")
            for half in range(2):
                kt = 2 * ktp + half
                nc.tensor.matmul(
                    t[:, half * 512 : (half + 1) * 512],
                    KT[h][:, kt * 128 : (kt + 1) * 128],
                    QT[h][:, qsl],
                    start=True,
                    stop=True,
                )
            return t

        def stage_a(h, qb):
            """Scores + exp + denominator chunk-accumulate for (h, qb)."""
            qsl = slice(qb * 512, (qb + 1) * 512)
            accd = p2s.tile([128, 512], F32R, tag="accd", bufs=PRE + 1, name="accd")
            accp = p2s.tile([128, 512], F32R, tag="accp", bufs=PRE + 1, name="accp")
            ets = []
            cur = scores(h, qsl, 0)
            for ktp in range(8):
                nxt = scores(h, qsl, ktp + 1) if ktp < 7 else None
                et = p2s.tile([128, 1024], BF16, tag=f"et{ktp}", bufs=PRE, name=f"et{ktp}")
                nc.scalar.activation(
                    et[:],
                    cur[:],
                    mybir.ActivationFunctionType.Exp,
                    scale=SCALE,
                )
                if masked:
                    mtile = p2s.tile([128, 2, 512], F32, tag="mtile", bufs=3, name="mtile")
                    nc.scalar.dma_start(
                        mtile[:], mT_v[:, 2 * ktp : 2 * ktp + 2, qsl]
                    )
                    nc.vector.tensor_mul(
                        et[:],
                        et[:],
                        mtile[:].rearrange("p c q -> p (c q)"),
                    )
                if ktp < 3:
                    eng, acc = nc.gpsimd, accp
                elif ktp < 7:
                    eng, acc = nc.vector, accd
                else:
                    eng = None  # ktp7 summed by the PE den matmuls
                if eng is not None:
                    if ktp in (0, 3):
                        eng.tensor_add(acc[:], et[:, 0:512], et[:, 512:1024])
                    else:
                        eng.tensor_add(acc[:], acc[:], et[:, 0:512])
                        eng.tensor_add(acc[:], acc[:], et[:, 512:1024])
                ets.append(et)
                cur = nxt
            ET[(h, qb)] = (ets, accd, accp)

        def stage_b(h, qb):
            """AV matmuls + denominator reduce + normalize into aoT."""
            qsl = slice(qb * 512, (qb + 1) * 512)
            ets, accd, accp = ET.pop((h, qb))
            ps_av = PB[0].tile([128, 512], F32, tag="av", bufs=2, name="ps_av")
            for ktp in range(8):
                for half in range(2):
                    kt = 2 * ktp + half
                    esl = slice(half * 512, (half + 1) * 512)
                    nc.tensor.matmul(
                        ps_av[:],
                        V[kt][:, h * 128 : (h + 1) * 128],
                        ets[ktp][:, esl],
                        start=(kt == 0),
                        stop=(kt == ST - 1),
                    )
            # psd shares the pmix ring with psy: both slots' consumers
            # (recip / copies) are fast, so no slow cross-round coupling
            psd_t = PB[0].tile([128, 512], F32, tag="pmix", bufs=2, name="psd_t")
            psd = psd_t[0:1, :]
            nc.tensor.matmul(psd, t_ok[:], accd[:], start=True, stop=False)
            nc.tensor.matmul(psd, t_ok[:], accp[:], start=False, stop=False)
            nc.tensor.matmul(
                psd, t_okb[:], ets[7][:, 0:512], start=False, stop=False
            )
            nc.tensor.matmul(
                psd, t_okb[:], ets[7][:, 512:1024], start=False, stop=True
            )
            rs = p2s.tile([1, 512], F32, tag="rs", bufs=2, name="rs")
            nc.vector.reciprocal_approx_fast(rs[:], psd)
            bc = p2s.tile([128, 512], F32, tag="bc", bufs=2, name="bc")
            nc.gpsimd.partition_broadcast(bc[:], rs[:])
            nc.vector.tensor_mul(aoT[h][:, qsl], ps_av[:], bc[:])

        def out_proj_st(st, p3s):
            """Out-projection + store for one 128-row seq tile."""
            stsl = slice(st * 128, (st + 1) * 128)
            yt = p3s.tile([128, D], F32, tag="yt", bufs=2, name="yt")
            for dcb in range(4):
                dsl = slice(dcb * 512, (dcb + 1) * 512)
                psy = PB[0].tile([128, 512], F32, tag="pmix", bufs=2, name="psy")
                for fc in range(HPG):
                    nc.tensor.matmul(
                        psy[:],
                        aoT[fc][:, stsl],
                        WoS[fc][:, dsl],
                        start=(fc == 0),
                        stop=(fc == HPG - 1),
                    )
                if dcb < 3:
                    nc.vector.tensor_copy(yt[:, dsl], psy[:])
                else:
                    nc.scalar.copy(yt[:, dsl], psy[:])
            qeng = nc.sync if st % 2 == 0 else nc.gpsimd
            qeng.dma_start(y[stsl, 0:1024], yt[:, 0:1024])
            qeng.dma_start(y[stsl, 1024:2048], yt[:, 1024:2048])

        # ---- phase 1: projections (K pass, then Q+V per seq block) ----
        with tc.tile_pool(name="p1x", bufs=1) as p1x, \
             tc.tile_pool(name="p1w", bufs=1) as p1w, \
             tc.tile_pool(name="p1ps", bufs=1, space="PSUM") as p1ps:
            t_wk = p1w.tile([128, KC, FC], BF16, tag="wk")
            t_wq = p1w.tile([128, KC, FC], BF16, tag="wq")
            t_wv = p1w.tile([128, KC, FC], BF16, tag="wv")
            for wt, wv_ in ((t_wk, wkT_v), (t_wq, wqT_v), (t_wv, wvT_v)):
                for ck in range(4):
                    nc.sync.dma_start(
                        wt[:, 4 * ck : 4 * ck + 4, :],
                        wv_[:, 4 * ck : 4 * ck + 4, :],
                    )

            def xload(sb):
                ssl = slice(sb * 512, (sb + 1) * 512)
                halves = []
                for hh in range(2):
                    xb = p1x.tile(
                        [128, 8, 512], BF16, tag="xb", bufs=3, name="xb"
                    )
                    for xc, eng in ((0, nc.scalar), (1, nc.gpsimd)):
                        c0 = 8 * hh + 4 * xc
                        eng.dma_start(
                            xb[:, 4 * xc : 4 * xc + 4, :],
                            xT_v[:, c0 : c0 + 4, ssl],
                        )
                    halves.append(xb)
                return halves

            # K pass (no bias needed: softmax is row-shift invariant)
            for sb in range(SB):
                ssl = slice(sb * 512, (sb + 1) * 512)
                xb = xload(sb)
                ps = [
                    p1ps.tile([128, 512], F32, tag="p1", bufs=4, name=f"p1_{mt}")
                    for mt in range(HPG)
                ]
                for kc in range(KC):
                    for mt in range(HPG):
                        nc.tensor.matmul(
                            ps[mt][:],
                            t_wk[:, kc, mt * 128 : (mt + 1) * 128],
                            xb[kc // 8][:, kc % 8, :],
                            start=(kc == 0),
                            stop=(kc == KC - 1),
                        )
                for mt in range(HPG):
                    nc.scalar.copy(KT[mt][:, ssl], ps[mt][:])

            # Q + V pass, with prepaid A stages between blocks
            nc.gpsimd.dma_start(t_bqb[:], bqb[:])
            nc.gpsimd.dma_start(t_ok[:], ones_k[:])
            nc.gpsimd.dma_start(t_okb[:], ones_kb[:])
            for sb in range(SB):
                ssl = slice(sb * 512, (sb + 1) * 512)
                xb = xload(sb)
                ps = [
                    p1ps.tile([128, 512], F32, tag="p1", bufs=4, name=f"p1_{mt}")
                    for mt in range(HPG)
                ]
                for kc in range(KC):
                    for mt in range(HPG):
                        nc.tensor.matmul(
                            ps[mt][:],
                            t_wq[:, kc, mt * 128 : (mt + 1) * 128],
                            xb[kc // 8][:, kc % 8, :],
                            start=(kc == 0),
                            stop=(kc == KC - 1),
                        )
                for mt in range(HPG):
                    nc.vector.tensor_add(
                        QT[mt][:, ssl], ps[mt][:], t_bqb[:, mt, :]
                    )
                psv = [
                    p1ps.tile([128, 512], F32, tag="p1", bufs=4, name=f"p1v{j}")
                    for j in range(4)
                ]
                for kc in range(KC):
                    for j in range(4):
                        nc.tensor.matmul(
                            psv[j][:],
                            xb[kc // 8][:, kc % 8, j * 128 : (j + 1) * 128],
                            t_wv[:, kc, :],
                            start=(kc == 0),
                            stop=(kc == KC - 1),
                        )
                for j in range(4):
                    nc.vector.tensor_copy(V[sb * 4 + j][:], psv[j][:])
                if sb < PRE:
                    stage_a(sb, 0)

        # ---- phase 2+3: pipelined attention + out-projection ----
        with tc.tile_pool(name="p23", bufs=1) as p23, \
             tc.tile_pool(name="p23ps", bufs=1, space="PSUM") as p23ps, \
             tc.tile_pool(name="p3s", bufs=1) as p3s:
            PB[0] = p23ps
            aoT.extend(
                p23.tile([128, S], BF16, tag=f"ao{h}", name=f"ao{h}")
                for h in range(HPG)
            )
            WoS.extend(
                p23.tile([128, D], BF16, tag=f"wo{fc}", name=f"wo{fc}")
                for fc in range(HPG)
            )
            for fc in range(HPG):
                nc.sync.dma_start(WoS[fc][:], woT_v[:, fc, :])

            groups = [(h, qb) for qb in range(QB) for h in range(HPG)]
            for i, g in enumerate(groups):
                stage_b(*g)
                if i + PRE < len(groups):
                    stage_a(*groups[i + PRE])
                # one out-projection seq tile per round, one q-block behind
                # the B stages so the normalize chain is off the PE's path
                if i >= 4:
                    out_proj_st(i - 4, p3s)
            for st in range(12, ST):
                out_proj_st(st, p3s)

    nc.finalize()
    return nc


def kernel(x, mask, Wq, bq, Wk, bk, Wv, bv, Wo, bo):
    x = np.asarray(x, dtype=np.float32)
    mask = np.asarray(mask)
    Wq, bq = np.asarray(Wq, np.float32), np.asarray(bq, np.float32)
    Wk = np.asarray(Wk, np.float32)
    Wv, bv = np.asarray(Wv, np.float32), np.asarray(bv, np.float32)
    Wo, bo = np.asarray(Wo, np.float32), np.asarray(bo, np.float32)

    masked = bool((mask == 0).any())
    key = masked
    if key not in _cache:
        _cache[key] = _build(masked)
    nc = _cache[key]

    xTb = [np.ascontiguousarray(x[b].T).astype(NPBF16) for b in range(B)]
    mTb = None
    if masked:
        mTb = [
            np.ascontiguousarray((mask[b, 0] != 0).T.astype(np.float32))
            for b in range(B)
        ]
    ok = np.ones((128, 1), np.float32)

    in_maps = []
    for c in range(8):
        g, b = c % G, c // G
        gs = slice(g * FC, (g + 1) * FC)
        bq2 = np.ascontiguousarray(bq[gs].reshape(HPG, 128).T)
        m = {
            "xT": xTb[b],
            "wqT": np.ascontiguousarray(Wq[gs].T).astype(NPBF16),
            "wkT": np.ascontiguousarray(Wk[gs].T).astype(NPBF16),
            "wvT": np.ascontiguousarray(Wv[gs].T).astype(NPBF16),
            "woT": np.ascontiguousarray(Wo[:, gs].T).astype(NPBF16),
            "bqb": np.ascontiguousarray(
                np.broadcast_to(bq2[:, :, None], (128, HPG, 512))
            ).astype(np.float32),
            "ones_k": ok,
            "ones_kb": ok.astype(NPBF16),
        }
        if masked:
            m["mT"] = mTb[b]
        in_maps.append(m)

    res = run_bass_kernel_spmd(
        nc, in_maps, core_ids=list(range(8)), trace=PROFILE
    )
    if PROFILE:
        LAST["exec_time_ns"] = res.exec_time_ns
        LAST["profile_json"] = res.profile_json
        LAST["trace"] = res.instructions_and_trace

    # host-side bias: bo plus the folded V bias (rows of attn sum to 1)
    bias = bo.astype(np.float64) + bv.astype(np.float64) @ Wo.T.astype(
        np.float64
    )
    out = np.empty((B, S, D), np.float32)
    for b in range(B):
        acc = res.results[4 * b]["y"].astype(np.float64)
        for g in range(1, G):
            acc += res.results[4 * b + g]["y"]
        out[b] = (acc + bias).astype(np.float32)
    return out
